# revision 23
# baseline (speedup 1.0000x reference)
"""Two-layer GAT (KeypointGraph) on 8 Trainium2 NeuronCores.

Strategy (x-space aggregation, dst-sharded, window-pipelined):
 - GAT algebra: out[d] = (1/4)·Σ_h (Σ_e α_eh x[src_e]) @ W_h + b — the linear
   transform commutes with the α-weighted aggregation, so each core aggregates
   256-wide x rows (not 1024-wide h rows) and applies W once per 128-dst
   window. No replicated X@W, no H table round trip; gather traffic drops 4x.
 - Host: add self-loops, balance dst nodes into 8 cores x 9 windows of 128
   dst slots (window 8 half-filled); per (core,window) edges padded to 128-edge
   tiles; one-hot med/mde per tile (bf16); gather table TAB[n,260] = [x | x@Wa_src]
   (bf16); per-window dst attention ADW = x@Wa_dst permuted to slots.
 - Device (one NEFF per layer run, SPMD on 8 cores), per window:
   per-tile indirect row gather (Pool) -> xg; psa matmul (mde^T @ adw) gives
   per-edge a_dst; logits = gathered a_src + psa -> Prelu -> Exp (Act);
   den[d,h] via med^T @ exw (PE, ~free); rec = 1/den; per-edge rec via
   mde^T @ rec; exwn = exw·rec (normalized weights); mx = med x exwn broadcast
   (DVE/Act split); two accumulating zT matmuls per tile:
   zT[c,(h,d)] += xg[:,chalf]^T @ mx  (PSUM, 2 banks); after the window:
   zT -> SBUF bf16, 8 accumulating matmuls vs W chunks -> out[d,256] (+bias).
 - Host between layers: x2 = relu(y1); rebuild TAB/ADW with layer-2 weights and
   rerun the same NEFF.
"""

import sys

sys.path.insert(0, "/opt/trn_rl_repo")

import numpy as np
import ml_dtypes

import concourse.bass as bass
import concourse.mybir as mybir
import concourse.tile as tile
from concourse.bass_utils import run_bass_kernel_spmd

BF16 = ml_dtypes.bfloat16

B, K, F = 512, 17, 256
N = B * K              # 8704
HEADS, C = 4, 256
NCORES = 8
NPC = N // NCORES      # 1088 dst nodes per core
NWIN = 17              # uniform 64-dst windows: halved mx/zT column space
DW = 64                # dst slots per window
CAPS = [DW] * NWIN
NPAD = 8832            # gather table rows (8704 real + pad)
PADROW = N             # gather index for padding edges
ROWW = 260             # table row: 256 x cols + 4 a_src cols (520B)
NEG_SLOPE = 0.2

_cache = {}


def _split_multiwaits(nc):
    """This image's walrus supports only ONE sync-wait command per
    instruction; hoist extra waits onto prepended same-engine NoOps."""
    for f in nc.m.functions:
        for blk in f.blocks:
            old = blk.instructions
            new = []
            changed = False
            for inst in old:
                si = inst.sync_info
                if si is not None and len(si.on_wait) > 1:
                    waits = list(si.on_wait)
                    for k, w in enumerate(waits[:-1]):
                        new.append(
                            mybir.InstNoOp(
                                name=f"{inst.name}_wsplit{k}",
                                engine=inst.engine,
                                sync_info=mybir.SyncInfo(on_wait=[w], on_update=[]),
                                bass_nofuse=True,
                            )
                        )
                    inst.sync_info = mybir.SyncInfo(
                        on_wait=[waits[-1]], on_update=list(si.on_update)
                    )
                    changed = True
                new.append(inst)
            if changed:
                blk.instructions = new


def _build_layer_nc(tw, mx_sched):
    """One GAT layer, SPMD over 8 cores. tw[w]: tiles per window."""
    nc = bass.Bass(num_devices=NCORES)
    dt = mybir.dt
    twmax = max(tw)

    TAB = nc.dram_tensor("tab", [NPAD, ROWW], dt.bfloat16, kind="ExternalInput")
    WG = nc.dram_tensor("wg", [2, 128, HEADS * C], dt.bfloat16, kind="ExternalInput")
    BIAS = nc.dram_tensor("bias", [1, C], dt.bfloat16, kind="ExternalInput")
    XD = nc.dram_tensor("xd", [NWIN, 128, ROWW], dt.bfloat16, kind="ExternalInput")
    SRC = nc.dram_tensor("src", [128, NWIN * twmax], dt.int32, kind="ExternalInput")
    ADW = nc.dram_tensor("adw", [128, NWIN * 4], dt.bfloat16, kind="ExternalInput")
    EPS = None
    MEDE = nc.dram_tensor(
        "mede", [NWIN, 128, twmax * 256], dt.bfloat16, kind="ExternalInput"
    )
    Y = nc.dram_tensor("y", [NWIN, DW, C], dt.float32, kind="ExternalOutput")

    with tile.TileContext(nc) as tc:
        with (
            tc.tile_pool(name="per", bufs=1) as per,
            tc.tile_pool(name="mw", bufs=4) as mw,
            tc.tile_pool(name="xg", bufs=4) as xgp,
            tc.tile_pool(name="sm", bufs=5) as sm,
            tc.tile_pool(name="mx", bufs=16) as mxp,
            tc.tile_pool(name="zs", bufs=3) as zs,
            tc.tile_pool(name="yt", bufs=3) as yt,
            tc.tile_pool(name="ppz", bufs=2, space="PSUM") as ppz,
            tc.tile_pool(name="pst", bufs=2, space="PSUM") as pst,
            tc.tile_pool(name="pot", bufs=2, space="PSUM") as pot,
        ):
            # ---- resident inputs ----
            wgs = []
            for k in range(2):
                w_ = per.tile([128, HEADS * C], dt.bfloat16, tag=f"wg{k}",
                              name=f"wg{k}")
                nc.scalar.dma_start(w_[:], WG[k])
                wgs.append(w_)
            bia = per.tile([1, C], dt.bfloat16, tag="bias")
            nc.scalar.dma_start(bia[:], BIAS[:])
            ones = per.tile([1, 128], dt.bfloat16, tag="ones")
            nc.vector.memset(ones[:], 1.0)
            epsb = per.tile([1, 4], dt.bfloat16, tag="epsb")
            nc.vector.memset(epsb[:], 1e-30)
            srcm = per.tile([128, NWIN * twmax], dt.int32, tag="srcm", name="srcm")
            nc.sync.dma_start(srcm[:], SRC[:, :])
            adwsb = per.tile([128, NWIN * 4], dt.bfloat16, tag="adw", name="adwsb")
            nc.sync.dma_start(adwsb[:], ADW[:, :])

            for w in range(NWIN):
                twn = tw[w]
                medw = mw.tile([128, twmax * 256], dt.bfloat16, tag="medw",
                               name=f"medw{w}")
                nc.sync.dma_start(medw[:, 0 : twn * 256], MEDE[w, :, 0 : twn * 256])

                # ---- per-tile indirect row gathers (Pool) ----
                xgw = xgp.tile([128, twmax * ROWW], dt.bfloat16, tag="xgw",
                               name=f"xgw{w}")
                sidxw = srcm[:, w * twmax : w * twmax + twn]
                # self-loop tile first: window dst rows direct from host table
                nc.sync.dma_start(xgw[:, 0:ROWW], XD[w])
                for t in range(1, twn):
                    nc.gpsimd.indirect_dma_start(
                        out=xgw[:, t * ROWW : (t + 1) * ROWW],
                        out_offset=None,
                        in_=TAB[:, :],
                        in_offset=bass.IndirectOffsetOnAxis(
                            ap=sidxw[:, t : t + 1], axis=0
                        ),
                    )

                # ---- strips: psa (per-edge a_dst), den, recp share one bank ----
                hp = tc.high_priority(offset=450)
                hp.__enter__()
                psT = pst.tile([128, 512], dt.float32, tag="pst", name=f"pst{w}")
                for t in range(twn):
                    nc.tensor.matmul(
                        psT[:, 4 * t : 4 * t + 4],
                        lhsT=medw[:, 256 * t + 128 : 256 * t + 256],
                        rhs=adwsb[:, 4 * w : 4 * w + 4],
                        start=True,
                        stop=True,
                    )

                # logits: eff = a_src(gathered) + psa; Prelu; Exp
                # split into two chunks so chunk 1 runs during chunk 2 gathers
                eff = sm.tile([128, 4 * twmax], dt.float32, tag="eff",
                              name=f"eff{w}")
                efl = sm.tile([128, 4 * twmax], dt.float32, tag="efl",
                              name=f"efl{w}")
                exwf = sm.tile([128, 4 * twmax], dt.float32, tag="exwf",
                               name=f"exwf{w}")
                exwb = sm.tile([128, 4 * twmax], dt.bfloat16, tag="exwb",
                               name=f"exwb{w}")
                tmid = (twn + 1) // 2
                for lo, hi in ((0, tmid), (tmid, twn - 1), (twn - 1, twn)):
                    nt = hi - lo
                    gv = xgw[:, lo * ROWW : hi * ROWW].rearrange(
                        "p (t c) -> p t c", t=nt, c=ROWW
                    )[:, :, 256:260]
                    nc.vector.tensor_add(
                        eff[:, 4 * lo : 4 * hi].rearrange(
                            "p (t c) -> p t c", t=nt, c=4),
                        gv,
                        psT[:, 4 * lo : 4 * hi].rearrange(
                            "p (t c) -> p t c", t=nt, c=4),
                    )
                    nc.scalar.activation(
                        efl[:, 4 * lo : 4 * hi], eff[:, 4 * lo : 4 * hi],
                        mybir.ActivationFunctionType.Prelu, alpha=NEG_SLOPE,
                    )
                    nc.scalar.activation(
                        exwf[:, 4 * lo : 4 * hi], efl[:, 4 * lo : 4 * hi],
                        mybir.ActivationFunctionType.Exp,
                    )
                    nc.scalar.copy(
                        exwb[:, 4 * lo : 4 * hi], exwf[:, 4 * lo : 4 * hi])

                # den[d,h] = sum_e exw (+eps so unused PSUM rows stay finite)
                nc.tensor.matmul(
                    psT[:, 96:100], lhsT=ones[:], rhs=epsb[:],
                    start=True, stop=False,
                )
                for t in range(twn):
                    nc.tensor.matmul(
                        psT[:, 96:100],
                        lhsT=medw[:, 256 * t : 256 * t + 128],
                        rhs=exwb[:, 4 * t : 4 * t + 4],
                        start=False,
                        stop=(t == twn - 1),
                    )
                recb = sm.tile([128, 4], dt.bfloat16, tag="recb", name=f"recb{w}")
                with nc.allow_low_precision(reason="1/den to bf16 matmul rhs"):
                    nc.vector.reciprocal(recb[:], psT[:, 96:100])
                for t in range(twn):
                    nc.tensor.matmul(
                        psT[:, 128 + 4 * t : 128 + 4 * t + 4],
                        lhsT=medw[:, 256 * t + 128 : 256 * t + 256],
                        rhs=recb[:],
                        start=True,
                        stop=True,
                    )
                exwnf = sm.tile([128, 4 * twmax], dt.float32, tag="exwnf",
                                name=f"exwnf{w}")
                nc.vector.tensor_mul(
                    exwnf[:, 0 : 4 * twn], exwf[:, 0 : 4 * twn],
                    psT[:, 128 : 128 + 4 * twn],
                )
                hp.__exit__(None, None, None)

                # ---- weighted aggregation in x-space ----
                zth = [
                    ppz.tile([128, 256], dt.float32, tag=f"zt{ch}",
                             name=f"zt{ch}_{w}")
                    for ch in range(2)
                ]
                for t in range(twn):
                    first = t == 0
                    last = t == twn - 1
                    mx = mxp.tile([128, 256], dt.bfloat16, tag="mx",
                                  name=f"mx_{w}_{t}")
                    if mx_sched[w][t] == 0:
                        # DVE: per-head scalar-ptr multiplies (2x fast path)
                        for h in range(HEADS):
                            nc.vector.tensor_scalar_mul(
                                mx[:, DW * h : DW * (h + 1)],
                                medw[:, 256 * t : 256 * t + DW],
                                exwnf[:, 4 * t + h : 4 * t + h + 1],
                            )
                    else:
                        # Act: per-head scalar-scale copies
                        for h in range(HEADS):
                            nc.scalar.mul(
                                mx[:, DW * h : DW * (h + 1)],
                                medw[:, 256 * t : 256 * t + DW],
                                exwnf[:, 4 * t + h : 4 * t + h + 1],
                            )
                    for ch in range(2):
                        nc.tensor.matmul(
                            zth[ch][:],
                            lhsT=xgw[:, t * ROWW + 128 * ch : t * ROWW + 128 * ch + 128],
                            rhs=mx[:],
                            start=first,
                            stop=last,
                        )

                # ---- per-window transform: out = sum_h z_h @ W_h/4 + b ----
                hp2 = tc.high_priority(offset=150)
                hp2.__enter__()
                zsa = zs.tile([128, 512], dt.bfloat16, tag="zsa", name=f"zsa{w}")
                nc.scalar.copy(zsa[:, 0:256], zth[0][:])
                nc.vector.tensor_copy(zsa[:, 256:512], zth[1][:])
                outw = pot.tile([DW, C], dt.float32, tag="outw", name=f"outw{w}")
                nc.tensor.matmul(
                    outw[:], lhsT=ones[:, 0:DW], rhs=bia[:], start=True, stop=False,
                )
                for ch in range(2):
                    for h in range(HEADS):
                        nc.tensor.matmul(
                            outw[:],
                            lhsT=zsa[:, 256 * ch + DW * h : 256 * ch + DW * (h + 1)],
                            rhs=wgs[ch][:, C * h : C * (h + 1)],
                            start=False,
                            stop=(h == HEADS - 1 and ch == 1),
                        )
                yacc = yt.tile([DW, C], dt.float32, tag="yacc", name=f"yacc{w}")
                nc.scalar.copy(yacc[:], outw[:])
                nc.sync.dma_start(Y[w], yacc[:])
                hp2.__exit__(None, None, None)

    _split_multiwaits(nc)
    return nc


def _host_prep(edge_index):
    ei = np.asarray(edge_index).astype(np.int64)
    loop = np.arange(N, dtype=np.int64)
    src = np.concatenate([ei[0], loop])
    dst = np.concatenate([ei[1], loop])

    # ---- balance dsts into (core, window) buckets by total degree ----
    NBK = NCORES * NWIN
    deg = np.bincount(dst, minlength=N).astype(np.int64)
    cap = np.array([CAPS[b % NWIN] for b in range(NBK)], np.int64)
    targ = deg.sum() / (N / 128.0)
    targ_b = targ * (cap / 128.0)
    order = np.argsort(-deg, kind="stable")
    L = np.zeros(NBK, np.float64)
    nfill = np.zeros(NBK, np.int64)
    bid = np.zeros(N, np.int64)
    for d in order:
        score = (L + deg[d]) - targ_b
        score[nfill >= cap] = np.inf
        b = int(np.argmin(score))
        L[b] += deg[d]
        bid[d] = b
        nfill[b] += 1

    # swap refinement: drive every bucket load to <= ceil-target so tiles/window
    # hit the minimum (full buckets 17*128 edges, half bucket 9*128-64)
    members = [list(np.where(bid == b)[0]) for b in range(NBK)]
    # total-degree limits so gathered (non-self) tiles hit the minimum count
    limit = np.full(NBK, 8 * 128 + DW, np.float64)
    for _ in range(5000):
        over = L - limit
        b1 = int(np.argmax(over))
        need = over[b1]
        if need <= 0:
            break
        m1 = members[b1]
        deg1 = deg[np.array(m1)]
        best = None
        for b2 in np.argsort(over)[:8]:
            b2 = int(b2)
            if b2 == b1 or over[b2] >= 0:
                continue
            m2 = members[b2]
            deg2 = deg[np.array(m2)]
            room = -over[b2]
            # delta in [1, min(need ceil-slack?, room)]; aim delta ~= need
            dmat = deg1[:, None] - deg2[None, :]
            ok = (dmat > 0) & (dmat <= room)
            if not ok.any():
                continue
            dm = np.where(ok, np.abs(dmat - need), np.inf)
            i, k = np.unravel_index(int(dm.argmin()), dm.shape)
            cand = (float(dm[i, k]), int(dmat[i, k]), b2, int(m1[i]), int(m2[k]))
            if best is None or cand[0] < best[0]:
                best = cand
        if best is None:
            break
        _, delta, b2, d1, d2 = best
        members[b1].remove(d1)
        members[b2].remove(d2)
        members[b1].append(d2)
        members[b2].append(d1)
        bid[d1], bid[d2] = b2, b1
        L[b1] -= delta
        L[b2] += delta

    pj = np.zeros(N, np.int64)
    pw = np.zeros(N, np.int64)
    pslot = np.zeros(N, np.int64)
    for b in range(NBK):
        for s, d in enumerate(members[b]):
            pj[d] = b // NWIN
            pw[d] = b % NWIN
            pslot[d] = s

    # only the E original edges go through the gather path; the N explicit
    # self-loops become one identity tile per window fed by a direct DMA
    nsrc = src[: len(ei[0])]
    ndst = dst[: len(ei[0])]
    ecore_n = pj[ndst]
    ewin_n = pw[ndst]
    dstw_n = pslot[ndst]
    cnt = np.zeros((NCORES, NWIN), np.int64)
    for j in range(NCORES):
        m = ecore_n == j
        for w in range(NWIN):
            cnt[j, w] = int((m & (ewin_n == w)).sum())
    tw = [int(np.ceil(cnt[:, w].max() / 128)) + 1 for w in range(NWIN)]
    twmax = max(tw)
    T = sum(tw)

    srcw = np.full((NCORES, NWIN, 128, twmax), PADROW, np.int32)
    dstwin = np.full((NCORES, NWIN, 128, twmax), -1, np.int64)
    adix = np.full((NCORES, NWIN, 128), PADROW, np.int32)
    dstid = np.arange(N, dtype=np.int64)
    adix[pj, pw, pslot] = dstid
    for j in range(NCORES):
        m = ecore_n == j
        for w in range(NWIN):
            mw_ = m & (ewin_n == w)
            s = nsrc[mw_]
            d = dstw_n[mw_]
            cnte = len(s)
            es = np.arange(cnte) + 128  # gathered tiles start at tile 1
            srcw[j, w, es % 128, es // 128] = s.astype(np.int32)
            dstwin[j, w, es % 128, es // 128] = d
            # identity self tile (tile 0) over the window's DW dst slots
            dstwin[j, w, 0:DW, 0] = np.arange(DW)

    iota = np.arange(128)
    med = (dstwin[..., None] == iota[None, None, None, None, :]).astype(BF16)
    mde = med.transpose(0, 1, 4, 3, 2).copy()
    mede = np.empty((NCORES, NWIN, 128, twmax, 256), BF16)
    mede[..., 0:128] = med
    mede[..., 128:256] = mde
    mede = mede.reshape(NCORES, NWIN, 128, twmax * 256).copy()

    srcw = srcw.transpose(0, 2, 1, 3).reshape(NCORES, 128, NWIN * twmax).copy()
    return tw, T, srcw, mede, adix, (pj, pw, pslot)


def _fold_attn(W, a):
    """Wa = W @ a per head: [F_in, HEADS]."""
    W64 = np.asarray(W, np.float64)
    A = np.asarray(a, np.float64)
    Wh = W64.reshape(W64.shape[0], HEADS, C)
    return (Wh * A[None]).sum(-1)  # [F_in, HEADS]


def _layer_inputs(x, Wl, a_src, a_dst, bias, adix, placement):
    """Host-side per-layer tensors: TAB, ADW, WG, BIAS, XD."""
    pj, pw, pslot = placement
    x64 = np.asarray(x, np.float64)
    asrc = x64 @ _fold_attn(Wl, a_src)          # [N, 4]
    adst = x64 @ _fold_attn(Wl, a_dst)          # [N, 4]
    tab = np.zeros((NPAD, ROWW), np.float32)
    tab[:N, 0:256] = np.asarray(x, np.float32)
    tab[:N, 256:260] = asrc
    tabb = tab.astype(BF16)
    xd = tabb[adix]                              # [NC, NWIN, 128, ROWW]
    adw = np.zeros((NCORES, NWIN, 128, 4), np.float32)
    adw[pj, pw, pslot] = adst
    adw = adw.transpose(0, 2, 1, 3).reshape(NCORES, 128, NWIN * 4).astype(BF16)
    wg = (np.asarray(Wl, np.float64) * 0.25).astype(BF16).reshape(2, 128, HEADS * C)
    bias_b = np.asarray(bias, np.float32)[None, :].astype(BF16)
    return tabb, adw, wg, bias_b, xd


def _run_layer(nc, tabb, adw, wg, bias_b, xd, srcw, mede, placement):
    in_maps = []
    for j in range(NCORES):
        in_maps.append(
            {
                "tab": tabb,
                "wg": wg,
                "bias": bias_b,
                "xd": xd[j],
                "src": srcw[j],
                "adw": adw[j],
                "mede": mede[j],
            }
        )
    res = run_bass_kernel_spmd(nc, in_maps, core_ids=list(range(NCORES)))
    pj, pw, pslot = placement
    yall = np.stack([res.results[j]["y"] for j in range(NCORES)])  # [NC,NWIN,DW,C]
    y = yall[pj, pw, pslot].astype(np.float32)
    return y


def _mx_schedule(tw):
    """Per (window, tile) engine for the mx broadcast: 0=DVE, 1=Act."""
    return [[1 if t == 4 else 0 for t in range(tw[w])] for w in range(NWIN)]


def kernel(kpt_feature, edge_index, W1, a_src1, a_dst1, b1, W2, a_src2, a_dst2, b2):
    key = "k"
    if key not in _cache:
        tw, T, srcw, mede, adix, placement = _host_prep(edge_index)
        nc = _build_layer_nc(tw, _mx_schedule(tw))
        _cache[key] = (nc, tw, T, srcw, mede, adix, placement)
    nc, tw, T, srcw, mede, adix, placement = _cache[key]

    x1 = np.asarray(kpt_feature, np.float32).reshape(N, F)
    y1 = _run_layer(
        nc, *_layer_inputs(x1, W1, a_src1, a_dst1, b1, adix, placement),
        srcw, mede, placement,
    )
    x2 = np.maximum(y1, 0.0)
    y2 = _run_layer(
        nc, *_layer_inputs(x2, W2, a_src2, a_dst2, b2, adix, placement),
        srcw, mede, placement,
    )
    return y2.reshape(B, K, F).astype(np.float32)
